# revision 45
# baseline (speedup 1.0000x reference)
"""Trainium2 Bass kernel for nn_EncoderBlock (dense transformer encoder block).

Strategy: pure data parallelism — batch B=8 across the 8 NeuronCores, one
batch element per core. No collectives. Per core:

  LN1 (bn_stats pairs over a bf16 x copy; DVE-only rsqrt via Taylor seed +
  Newton) -> q = n@wqT+bq (kh=vh=qh, reproducing the reference's q-reuse
  bug); attention per head pair, software-pipelined so scores(i+1) matmuls
  run under exp(i): S = q^T q (symmetric), E = exp(S/8 - 20);
  ctx AND Z come from ONE matmul per head: the stationary operand is the
  per-head slot [ones(64) | q(64)], so psum rows 0:64 hold Z (replicated)
  and rows 64:128 hold unnormalized ctx^T; normalization is one
  reciprocal_approx_fast + one cross-base tensor_mul per head (PSUM input
  may carry a different base partition than the SBUF operands — verified
  on HW).  The softmax exps alternate between ScalarE (exact ACT spline)
  and the DVE (Schraudolph bit-trick: i16 = round(x*128/ln2 + 127*128)
  bitcast to bf16, ~±3% on individual weights, cancels in the softmax
  ratio; measured no effect on final rel-err) so neither engine gates the
  scores->ctx pipeline; wo projection + residual; LN2; ReLU FFN (d_ff=4096)
  streamed from HBM on contention-separated queues; residual; out.

All transposes (n1T / qo / n2Th) are XBAR DMA-transposes — the PE does
only matmuls.  Matmuls run in bf16 (fp32 accumulation in PSUM); layernorm
statistics, softmax sums and the residual stream stay fp32.
"""

import sys

sys.path.insert(0, "/opt/trn_rl_repo")

import math

import numpy as np
import ml_dtypes
from contextlib import ExitStack

import concourse.bass as bass
import concourse.tile as tile
from concourse import bacc, mybir
from concourse import bass_utils
from concourse.bass import ts, ds
from concourse.masks import make_identity

BF = mybir.dt.bfloat16
F32 = mybir.dt.float32
I16 = mybir.dt.int16
AF = mybir.ActivationFunctionType
OP = mybir.AluOpType
AX = mybir.AxisListType

P = 128
S = 1024          # sequence length per core
D = 1024          # d_model
H = 16            # heads
DK = 64           # head dim
DFF = 4096
NB = 8            # batch = number of cores
SC = S // P       # 8 sequence chunks
DC = D // P       # 8 feature chunks
FC = DFF // P     # 32 ff chunks
EPS = 1e-6
EXP_SHIFT = -20.0  # constant shift inside exp; cancels in softmax ratio
# Schraudolph exp on the DVE: bf16 y with bits int16(round(x*128/ln2 +
# 127*128 - C)) satisfies y ~= exp(x)*(1 +/- 3%); C centers the one-sided
# piecewise-linear error.  Fold in the 1/8 score scale and EXP_SHIFT.
SCH_A = 0.125 * 128.0 / math.log(2.0)
SCH_B = 127.0 * 128.0 + EXP_SHIFT * (128.0 / math.log(2.0)) - 5.68

last_exec_time_ns = None


def _emit_layernorm(nc, small, xt, n_out, alpha, beta, idx, chunks,
                    apply_eng=None):
    """LN with Bessel-corrected std (ddof=1), matching torch/jax reference:
    n = (x - mu)/(std + eps)*alpha + beta.  xt [P,*,D] f32 indexed by `chunks`,
    n_out [P,len(chunks),D] bf16 indexed locally.
    Stats per token live on partitions; mean+var via one bn_stats pass (DVE
    only — keeps ScalarE free for the softmax exps)."""
    chunks = list(chunks)
    nch = len(chunks)
    BSD = nc.vector.BN_STATS_DIM
    bsf = 512  # BN_STATS_FMAX; D = 2 subgroups of 512
    nsub = D // bsf
    bst = small.tile([P, nch, nsub, BSD], F32, name=f"ln{idx}_bst")
    mv = small.tile([P, nch, 2], F32, name=f"ln{idx}_mv")
    var = small.tile([P, nch], F32, name=f"ln{idx}_var")
    tmp = small.tile([P, nch], F32, name=f"ln{idx}_tmp")
    tcoef = small.tile([P, nch], F32, name=f"ln{idx}_t")
    ucoef = small.tile([P, nch], F32, name=f"ln{idx}_u")

    for i, sc in enumerate(chunks):
        xv = xt[:, sc].rearrange("p (ns f) -> p ns f", ns=nsub)
        for sg in range(nsub):
            nc.vector.bn_stats(bst[:, i, sg], xv[:, sg])
        nc.vector.bn_aggr(mv[:, i], bst[:, i])
    mu = mv[:, :, 0]  # strided [P, nch] views
    # var (ddof=1)
    nc.vector.tensor_scalar_mul(var[:], mv[:, :, 1], float(D) / (D - 1))
    # 1/(std+eps) ~= rsqrt(var) (eps=1e-6 << std~1, relative error ~1e-6),
    # computed entirely on the DVE: LN variances concentrate near 1, so a
    # Taylor seed sqrt(r)~=1+(r-1)/2-(r-1)^2/8 off r=1/var (approx_fast)
    # plus two multiply-only Newton steps y*(1.5-0.5*var*y^2) reaches
    # ~1e-6 rel.  No ScalarE Sqrt -> no ACT-table switches anywhere.
    r = small.tile([P, nch], F32, name=f"ln{idx}_r")
    t2 = small.tile([P, nch], F32, name=f"ln{idx}_t2")
    y = small.tile([P, nch], F32, name=f"ln{idx}_y")
    nc.vector.reciprocal(r[:], var[:])
    nc.vector.tensor_scalar(tmp[:], r[:], 1.0, None, OP.subtract)  # t=r-1
    nc.vector.tensor_mul(t2[:], tmp[:], tmp[:])
    nc.vector.tensor_scalar(tmp[:], tmp[:], 0.5, 1.0, OP.mult, OP.add)
    nc.vector.scalar_tensor_tensor(y[:], t2[:], -0.125, tmp[:],
                                   OP.mult, OP.add)                # y0
    for _ in range(2):
        nc.vector.tensor_mul(t2[:], y[:], y[:])
        nc.vector.tensor_mul(t2[:], t2[:], var[:])
        nc.vector.tensor_scalar(t2[:], t2[:], -0.5, 1.5, OP.mult, OP.add)
        nc.vector.tensor_mul(y[:], y[:], t2[:])
    nc.vector.tensor_scalar_mul(tcoef[:], y[:], float(alpha))
    nc.vector.tensor_mul(tmp[:], mu, tcoef[:])
    nc.vector.tensor_scalar(ucoef[:], tmp[:], -1.0, float(beta), OP.mult, OP.add)
    apply_eng = apply_eng or nc.vector
    for i, sc in enumerate(chunks):
        if apply_eng is nc.scalar:
            # ACT affine: Identity(x*tcoef + ucoef) — rides the idle
            # ScalarE in the wo windows
            apply_eng.activation(
                n_out[:, i], xt[:, sc], mybir.ActivationFunctionType.Identity,
                bias=ucoef[:, ds(i, 1)], scale=tcoef[:, ds(i, 1)],
            )
        else:
            apply_eng.tensor_scalar(
                n_out[:, i], xt[:, sc], tcoef[:, ds(i, 1)], ucoef[:, ds(i, 1)],
                OP.mult, OP.add,
            )


def _emit_transpose(nc, pool, dst, src, ident, ca_range=range(8),
                    copy_engs=None):
    """dst = 8x8 block transpose of src; both [P, 8, 1024] (bf16).
    PSUM evacuations rotate over copy_engs (default Vector/Scalar split)."""
    for ca in ca_range:
        for cb in range(8):
            pt = pool.tile([P, P], src.dtype, tag="tp", bufs=4, name="tp")
            nc.tensor.transpose(pt[:], src[:, ca, ts(cb, P)], ident[:])
            engs = copy_engs or [nc.vector, nc.scalar]
            eng = engs[cb % len(engs)]
            if eng is nc.scalar:
                eng.copy(dst[:, cb, ts(ca, P)], pt[:])
            else:
                eng.tensor_copy(dst[:, cb, ts(ca, P)], pt[:])


def build_program(ln1a, ln1b, ln2a, ln2b, mask_all_ones):
    import os
    phase_stop = int(os.environ.get("BASSK_PHASE", "9"))
    nc = bacc.Bacc("TRN2", target_bir_lowering=False, debug=False)

    x_d = nc.dram_tensor("x", (S, D), F32, kind="ExternalInput").ap()
    xbf_d = nc.dram_tensor("x_bf", (S, D), BF, kind="ExternalInput").ap()
    wqT_d = nc.dram_tensor("wqT", (D, D), BF, kind="ExternalInput").ap()
    woT_d = nc.dram_tensor("woT", (D, D), BF, kind="ExternalInput").ap()
    w1T_d = nc.dram_tensor("w1T", (D, DFF), BF, kind="ExternalInput").ap()
    w2T_d = nc.dram_tensor("w2T", (DFF, D), BF, kind="ExternalInput").ap()
    bq_d = nc.dram_tensor("bq_v", (P, DC), F32, kind="ExternalInput").ap()
    b1_d = nc.dram_tensor("b1_v", (P, FC), F32, kind="ExternalInput").ap()
    bo_d = nc.dram_tensor("bo_rep", (P, D), BF, kind="ExternalInput").ap()
    b2_d = nc.dram_tensor("b2_rep", (P, D), BF, kind="ExternalInput").ap()
    if not mask_all_ones:
        m01_d = nc.dram_tensor("m01_v", (P, SC), F32, kind="ExternalInput").ap()
    out_d = nc.dram_tensor("out", (S, D), F32, kind="ExternalOutput").ap()

    x_r = x_d.rearrange("(sc p) d -> sc p d", p=P)
    xbf_r = xbf_d.rearrange("(sc p) d -> sc p d", p=P)
    wqT_r = wqT_d.rearrange("(kc p) o -> kc p o", p=P)
    woT_r = woT_d.rearrange("(oc p) d -> oc p d", p=P)
    w1_batched = w1T_d.rearrange("(dc p) f -> p dc f", p=P)
    w2_batched = w2T_d.rearrange("(fc p) d -> p fc d", p=P)
    out_r = out_d.rearrange("(sc p) d -> sc p d", p=P)

    with tile.TileContext(nc) as tc, ExitStack() as st:
        arena = st.enter_context(tc.tile_pool(name="arena", bufs=1))
        small = st.enter_context(tc.tile_pool(name="small", bufs=1))

        # ---- constants ----
        ident_b = small.tile([P, P], BF, name="ident_b")
        make_identity(nc, ident_b[:])
        ebias = small.tile([P, 1], F32, name="ebias")
        nc.gpsimd.memset(ebias[:], EXP_SHIFT)
        # warm the GpSimd tensor-op ucode library at t=0: its ~6us IRAM
        # load otherwise lands right on the LN1-apply critical path
        gpwarm = small.tile([P, 1], F32, name="gpwarm")
        nc.gpsimd.tensor_scalar(gpwarm[:], ebias[:], 0.0, None, OP.mult)
        bq_sb = small.tile([P, DC], F32, name="bq_sb")
        b1_sb = small.tile([P, FC], F32, name="b1_sb")
        bo_rep = small.tile([P, D], BF, name="bo_rep")
        b2_rep = small.tile([P, D], BF, name="b2_rep")
        if not mask_all_ones:
            m01_sb = small.tile([P, SC], F32, name="m01_sb")
            nc.sync.dma_start(m01_sb[:], m01_d)

        def emit_bias_dmas():
            # biases are needed late; ScalarE has early-queue slack and
            # keeping them off Sync/GpSimd unblocks the n1T transposes and
            # the LN1 applies
            nc.scalar.dma_start(bq_sb[:], bq_d)
            nc.scalar.dma_start(b1_sb[:], b1_d)
            nc.scalar.dma_start(bo_rep[:], bo_d)
            nc.scalar.dma_start(b2_rep[:], b2_d)

        dma_engines = [nc.sync, nc.scalar, nc.gpsimd]
        # DMA issue queues that never carry softmax exps — weight streams
        # during the attention/FFN overlap must not block the Scalar queue
        # (DMA_DIRECT2D there waits on FFN semaphores ahead of exps).
        dma_quiet = [nc.sync, nc.gpsimd]

        # ---- phase A inputs ----
        # The startup is chip-HBM-contention-bound (all 8 cores load at
        # once), so LN1 reads a bf16 copy of x — half the bytes.  The f32 x
        # is only needed for the residual and is re-loaded later, off the
        # critical path.  GpSimd carries no weight DMAs so the LN applies
        # are not queued behind arrivals.
        xt = arena.tile([P, SC, D], BF, tag="xt_h1", name="xt")
        # keep GpSimd's early queue free for the LN1 applies
        x_eng = [nc.sync, nc.scalar, nc.sync, nc.scalar,
                 nc.sync, nc.scalar, nc.sync, nc.scalar]
        for sc in range(SC):
            x_eng[sc].dma_start(xt[:, sc], xbf_r[sc])
        qT = arena.tile([P, DC, S], BF, tag="qT", name="qT")  # [o%P, oc, s]
        # Fused ctx+Z stationary slots: qo[:, hp, c, h] = [ones(64)|q_h(64)]
        # so ONE matmul per (head, chunk) yields Z (psum rows 0:64,
        # replicated) and unnormalized ctx^T (rows 64:128).  Layout is
        # hp-major with c-stride == 2*h-stride so one DMA-transpose per oc
        # fills all 4 chunks x 2 heads (the (c,h) dims merge to a legal 3D
        # dst); the ones halves are memset once.
        qo = arena.tile([P, DC, SC, 2, P], BF, tag="qq_out", name="qo")
        n1 = arena.tile([P, SC, D], BF, tag="n1_ctx", name="n1")
        n1T = arena.tile([P, DC, S], BF, tag="n1T_woT", name="n1T")
        wq_sb = arena.tile([P, DC, D], BF, tag="wq_res1", name="wq_sb")
        for kc in range(DC):
            [nc.sync, nc.scalar][kc % 2].dma_start(wq_sb[:, kc], wqT_r[kc])
        emit_bias_dmas()

        # ================= phase A: LN1, q projection, transposes ============
        # LN1 split into halves so the first q-proj matmuls (which only read
        # n1T columns 0:512 = seq chunks 0-3) start while LN of chunks 4-7
        # still runs on the DVE.
        with tc.tile_pool(name="psA", bufs=1, space="PSUM") as psA:
            # HAM warm-up: dead transposes keep the PE clock-gate's busy
            # window active across the whole LN1 window (~15-30us of
            # HBM-contended arrivals), so q-proj and early attention run at
            # 2.4 GHz instead of starting cold at 1.2 GHz
            for _ in range(120):
                wup = psA.tile([P, P], BF, tag="tp", bufs=4, name="wup")
                nc.tensor.transpose(wup[:], ident_b[:], ident_b[:])
            # LN1 in chunk pairs: stats on Vector, applies on GpSimd, and
            # the n1 -> n1T transposes as XBAR DMA-transposes on Sync (one
            # [128,1024] DMA per chunk writes the whole strided slice) —
            # three disjoint queues, nothing blocks the next pair's stats,
            # and the PE does no transpose work at all.
            for pi in range(SC // 2):
                pr = range(2 * pi, 2 * pi + 2)
                _emit_layernorm(nc, small, xt, n1[:, ds(2 * pi, 2)],
                                ln1a, ln1b, f"1p{pi}", pr,
                                apply_eng=nc.gpsimd)
                # NB: all DMA-transposes must stay on ONE queue — the
                # transpose XBAR is a shared resource; driving it from two
                # queues concurrently corrupts the output (measured).
                for ca in pr:
                    nc.sync.dma_start_transpose(
                        n1T[:, :, ts(ca, P)], n1[:, ca])

            def qproj_oc(pool, tag, bufs, b, oc, evac_act=False):
                pbt = pool.tile([P, 512], F32, tag=tag, bufs=bufs,
                                name="qps")
                for kc in range(DC):
                    nc.tensor.matmul(
                        pbt[:], wq_sb[:, kc, ts(oc, P)],
                        n1T[:, kc, ds(512 * b, 512)],
                        start=(kc == 0), stop=(kc == DC - 1),
                    )
                if evac_act:
                    # prolog evacs ride ScalarE (Identity+bias) so the DVE
                    # keeps its attention exp budget
                    nc.scalar.add(
                        qT[:, oc, ds(512 * b, 512)], pbt[:],
                        bq_sb[:, ds(oc, 1)],
                    )
                else:
                    nc.vector.tensor_scalar(
                        qT[:, oc, ds(512 * b, 512)], pbt[:],
                        bq_sb[:, ds(oc, 1)], None, OP.add,
                    )

            for oc in range(DC):
                qproj_oc(psA, "qps", 4, 0, oc)
            # ones halves of the qo slots, in small chunks emitted after
            # the qproj evacs: the Tile priority heap then only runs them
            # in DVE idle slots (one 7us memset here measurably delayed the
            # whole LN1 -> qproj critical path)
            qo_ones = qo[:].rearrange("p a b c d -> p (a b c) d")[:, :, 0:DK]
            for oc in range(DC):
                nc.vector.memset(qo_ones[:, ds(16 * oc, 16)], 1.0)

            def qo_transpose(oc):
                # One [64,1024] XBAR DMA-transpose per head fills all 8
                # chunks of that head's qo slots: dst [p, c(8), d(64)] has
                # mid extent == transposed row blocks and last extent ==
                # transposed cols (the only shape the engine honors).
                # Requires qT[:, oc, :] complete (b0 AND b1).
                for hl in range(2):
                    nc.sync.dma_start_transpose(
                        qo[:, oc, :, hl, DK:P],
                        qT[ds(hl * DK, DK), oc, :])

            # q-proj b1 for oc 0,1 now (their qo slots are needed in
            # attn0's first iterations — too early for the spread-out
            # filler steps); the rest of b1 spreads across attn0
            n_pre = 2 if os.environ.get("BASSK_FA", "1") == "1" else DC
            for oc in range(n_pre):
                qproj_oc(psA, "qps", 4, 1, oc)
                qo_transpose(oc)

        def mk_fill_qproj(pool):
            # Spread q-proj b1 (oc 2..7) across attn0's iterations: one
            # 512-cycle contraction step per filler call keeps the PE queue
            # dense underneath the exp latency (no stall, HAM stays warm).
            st_ = {}
            steps = []
            for oc in range(2, DC):
                for kc in range(DC):
                    def step(oc=oc, kc=kc):
                        if kc == 0:
                            st_[oc] = pool.tile([P, 512], F32, tag="mixQ",
                                                bufs=2, name="qps1")
                        nc.tensor.matmul(
                            st_[oc][:], wq_sb[:, kc, ts(oc, P)],
                            n1T[:, kc, ds(512, 512)],
                            start=(kc == 0), stop=(kc == DC - 1),
                        )
                        if kc == DC - 1:
                            nc.scalar.add(
                                qT[:, oc, ds(512, 512)], st_[oc][:],
                                bq_sb[:, ds(oc, 1)])
                            qo_transpose(oc)
                    steps.append(step)
            done = [0]

            def filler(idx):
                target = min(len(steps), (idx + 1) * len(steps) // 50)
                while done[0] < target:
                    steps[done[0]]()
                    done[0] += 1
            return filler

        if phase_stop <= 1:
            for sc in range(SC):
                dt_ = arena.tile([P, D], F32, tag="dump", bufs=2, name="dump")
                nc.vector.tensor_copy(dt_[:], qT[:, sc])
                nc.sync.dma_start(out_r[sc], dt_[:])
            nc.compile()
            return nc
        if phase_stop == 2:
            for sc in range(SC):
                dt_ = arena.tile([P, D], F32, tag="dump", bufs=2, name="dump")
                nc.vector.tensor_copy(
                    dt_[:].rearrange("p (b c d) -> p b c d", b=4, c=2),
                    qo[:, sc // 2, (sc % 2) * 4:(sc % 2) * 4 + 4])
                nc.sync.dma_start(out_r[sc], dt_[:])
            nc.compile()
            return nc

        # persistent across the pipelined halves
        ctxT = arena.tile([P, DC, S], BF, tag="n1_ctx", name="ctxT")
        woT_sb = arena.tile([P, DC, D], BF, tag="n1T_woT", name="woT_sb")
        # anchor the woT stream behind qT so the scheduler cannot hoist
        # these 2MB of loads into the startup HBM window (x_bf/wq arrival
        # paces the whole LN1->qproj chain)
        nc.scalar.copy(woT_sb[0:1, 0, 0:1], qT[0:1, 0, 0:1])
        for oc in range(DC):
            nc.sync.dma_start(woT_sb[:, oc], woT_r[oc])
        res1 = arena.tile([P, SC, D], F32, tag="wq_res1", name="res1")
        out_sb = None

        # ============ attention / wo / LN2 / FFN pipelined by query halves ===
        #
        # The 128 softmax exps alternate ScalarE (exact) / DVE (Schraudolph
        # int16 bit-trick written through a bf16 bitcast view) so no single
        # engine gates the pipeline; the fused [ones|q] stationary gives
        # ctx and Z from one matmul per (head, chunk).

        def attn_half(psT, half, filler=None, look=1, sbufs=2,
                      exp_act_until=0):
            # Software-pipelined: scores(i+look) matmuls are EMITTED before
            # ctx(i) so the in-order PE queue runs them underneath exp(i);
            # `filler(idx)` injects independent PE work (q-proj b1 steps,
            # ffn1 chunks) between scores(i+look) and ctx(i) — the queue
            # stays dense so the exp latency never stalls the PE and the
            # HAM clock-gate stays at 2.4 GHz.  The exps split 5:3
            # ScalarE:DVE — the DVE also carries the normalize.
            iters = [(hp, c) for hp in range(H // 2) for c in range(SC)]
            state = {}

            def emit_scores(hp, c):
                sp = psT.tile([P, 1024], F32, tag="scp", bufs=sbufs,
                              name="scp")
                for hl in range(2):
                    lo = hl * DK
                    nc.tensor.matmul(
                        sp[:, ds(hl * 512, 512)],
                        qT[ds(lo, DK), hp, ts(c, P)],
                        qT[ds(lo, DK), hp, ds(512 * half, 512)],
                        start=True, stop=True,
                        tile_position=(lo, 0),
                    )
                return sp

            sp_pend = [emit_scores(*iters[i]) for i in range(look)]
            for idx, (hp, c) in enumerate(iters):
                sp = sp_pend.pop(0)
                if idx + look < len(iters):
                    sp_pend.append(emit_scores(*iters[idx + look]))
                if filler is not None:
                    filler(idx)
                if hp not in state:
                    state[hp] = psT.tile([P, 2 * 512], F32, tag="ctx2",
                                         bufs=1, name="ctx2")
                ctx2 = state[hp]
                # ec is consumed by the two fused matmuls of this iteration
                # only — a rolling buffer is enough pipeline depth
                ec = arena.tile([P, 2 * 512], BF, tag="EC", bufs=4,
                                name="ec")
                if idx < exp_act_until or (idx % 8) not in (1, 4, 6):
                    nc.scalar.activation(
                        ec[:], sp[:], AF.Exp, bias=ebias[:], scale=0.125,
                    )
                else:
                    nc.vector.tensor_scalar(
                        ec[:].bitcast(I16), sp[:], SCH_A, SCH_B,
                        OP.mult, OP.add,
                    )
                if not mask_all_ones:
                    nc.vector.tensor_scalar_mul(
                        ec[:], ec[:], m01_sb[:, ds(c, 1)],
                    )
                for hl in range(2):
                    nc.tensor.matmul(
                        ctx2[:, ds(hl * 512, 512)], qo[:, hp, c, hl],
                        ec[:, ds(hl * 512, 512)],
                        start=(c == 0), stop=(c == SC - 1),
                        skip_group_check=True,
                    )
                if c == SC - 1:
                    # Z sits on psum rows 0:64 (base 0 — required by
                    # reciprocal_approx_fast), ctx^T on rows 64:128; the
                    # muls read the PSUM ctx at base 64 against base-0 SBUF
                    # operands (HW-verified legal: only SBUF-SBUF operand
                    # pairs must share a base partition).
                    rz = arena.tile([DK, 2 * 512], F32, tag="rz", bufs=1,
                                    name="rz")
                    nc.vector.reciprocal_approx_fast(rz[:], ctx2[0:DK])
                    nc.vector.tensor_mul(
                        ctxT[0:DK, hp, ds(512 * half, 512)],
                        ctx2[DK:P, ds(0, 512)], rz[:, ds(0, 512)])
                    nc.vector.tensor_mul(
                        ctxT[DK:P, hp, ds(512 * half, 512)],
                        ctx2[DK:P, ds(512, 512)], rz[:, ds(512, 512)])
                    del state[hp]

        def wo_group(psW, sc, dh, bufs=2):
            if dh == 0:
                xre = arena.tile([P, D], F32, tag="xre", bufs=2, name="xre")
                wo_group.xre[sc] = xre
                # same anti-hoist anchor as woT: keep the 512KB f32 x loads
                # out of the startup window
                nc.scalar.copy(xre[0:1, 0:1], qT[0:1, 0, 0:1])
                nc.sync.dma_start(xre[:], x_r[sc])
                # precombine x + bo off the critical path so each wo PSUM
                # bank frees after a single add
                nc.gpsimd.tensor_add(xre[:], xre[:], bo_rep[:])
            xre = wo_group.xre[sc]
            wp = psW.tile([P, 512], F32, tag="mix", bufs=bufs, name="wops")
            for oc in range(DC):
                nc.tensor.matmul(
                    wp[:], ctxT[:, oc, ts(sc, P)],
                    woT_sb[:, oc, ds(512 * dh, 512)],
                    start=(oc == 0), stop=(oc == DC - 1),
                )
            nc.vector.tensor_add(
                res1[:, sc, ds(512 * dh, 512)], wp[:],
                xre[:, ds(512 * dh, 512)],
            )
        wo_group.xre = {}

        def wo_half(psW, half, bufs=2):
            for sl in range(SC // 2):
                for dh in range(2):
                    wo_group(psW, half * (SC // 2) + sl, dh, bufs=bufs)

        def ln2_half(psB, half):
            n2h = arena.tile([P, SC // 2, D], BF, tag="n2h", bufs=1,
                             name="n2h")
            chunks = range(half * (SC // 2), (half + 1) * (SC // 2))
            _emit_layernorm(nc, small, res1, n2h, ln2a, ln2b, f"2h{half}",
                            chunks, apply_eng=nc.scalar)
            n2Th = arena.tile([P, DC, 512], BF, tag="n2th", bufs=1,
                              name="n2Th")
            for ca in range(SC // 2):
                nc.sync.dma_start_transpose(n2Th[:, :, ts(ca, P)],
                                            n2h[:, ca])
            for sc in chunks:
                nc.gpsimd.tensor_add(res1[:, sc], res1[:, sc], b2_rep[:])
            return n2Th

        def ffn1_half(psB, wsp, half, n2Th):
            h1 = arena.tile([P, FC, 512], BF, tag="xt_h1", name="h1")
            # two queues hide the per-DMA completion latency.  Half 0 runs
            # concurrently with attention-half-1 exps, so its second queue
            # is Sync (Scalar would stall exps behind buffer-gated DMAs);
            # half 1 runs when Scalar is exp-free.
            w1q = [nc.gpsimd, nc.sync if half == 0 else nc.scalar]
            wts_t = {}

            def ens1(fc):
                if fc < FC and fc not in wts_t:
                    wts_t[fc] = wsp.tile([P, DC, P], BF, tag="w1s", bufs=3,
                                         name="w1s")
                    w1q[fc % 2].dma_start(wts_t[fc][:],
                                          w1_batched[:, :, ts(fc, P)])
            for fc in range(FC):
                ens1(fc)
                ens1(fc + 1)
                wts = wts_t[fc]
                fp = psB.tile([P, 512], F32, tag="mix", bufs=2, name="f1ps")
                for dc in range(DC):
                    nc.tensor.matmul(
                        fp[:], wts[:, dc], n2Th[:, dc, :],
                        start=(dc == 0), stop=(dc == DC - 1),
                    )
                nc.vector.tensor_scalar(
                    h1[:, fc], fp[:], b1_sb[:, ds(fc, 1)], 0.0,
                    OP.add, OP.max,
                )
            return h1

        def mk_fill_ffn1(psT, wsp, n2Th, start_iter=12):
            # ffn1(half 0) sliced into 2-matmul micro-ops and fed into
            # attn1's filler slots: the independent FFN work absorbs the
            # exp latency and keeps the PE at 2.4 GHz.  Starts a few
            # iterations in so ln2(0)'s DVE work and n2Th transposes clear
            # first.  w1 streams are issued one fc ahead of their matmuls.
            h1 = arena.tile([P, FC, 512], BF, tag="xt_h1", name="h1")
            st_ = {}
            w1q = [nc.gpsimd, nc.sync]

            def ensure_dma(fc):
                if fc < FC and fc not in st_:
                    wts = wsp.tile([P, DC, P], BF, tag="w1s", bufs=3,
                                   name="w1s")
                    w1q[fc % 2].dma_start(wts[:],
                                          w1_batched[:, :, ts(fc, P)])
                    st_[fc] = wts

            units = []
            for fc in range(FC):
                for u in range(4):
                    def unit(fc=fc, u=u):
                        if u == 0:
                            ensure_dma(fc)
                            ensure_dma(fc + 1)
                            st_[fc] = (st_[fc],
                                       psT.tile([P, 512], F32, tag="mixF",
                                                bufs=2, name="f1ps"))
                        wts, fp = st_[fc]
                        for dc in (2 * u, 2 * u + 1):
                            nc.tensor.matmul(
                                fp[:], wts[:, dc], n2Th[:, dc, :],
                                start=(dc == 0), stop=(dc == DC - 1),
                            )
                        if u == 3:
                            # evacs alternate ScalarE (Relu+bias via the
                            # ACT affine) / DVE so neither engine's exp
                            # budget is eaten
                            if fc % 2 == 0:
                                nc.scalar.activation(
                                    h1[:, fc], fp[:], AF.Relu,
                                    bias=b1_sb[:, ds(fc, 1)], scale=1.0)
                            else:
                                nc.vector.tensor_scalar(
                                    h1[:, fc], fp[:], b1_sb[:, ds(fc, 1)],
                                    0.0, OP.add, OP.max)
                    units.append(unit)
            done = [0]

            def filler(idx):
                if idx < start_iter:
                    return
                navail = 64 - start_iter - 2
                target = min(len(units),
                             (idx - start_iter + 1) * len(units) // navail)
                while done[0] < target:
                    units[done[0]]()
                    done[0] += 1

            def flush():
                while done[0] < len(units):
                    units[done[0]]()
                    done[0] += 1
            return h1, filler, flush

        def ffn2_half(psF2, wsp, half, h1, dhs=(0, 1), filler=None):
            nonlocal out_sb
            if out_sb is None:
                out_sb = arena.tile([P, SC, D], F32, tag="qq_out",
                                    name="out_sb")
            for dh in dhs:
                ops = [psF2.tile([P, 512], F32, tag="f2ps", bufs=6,
                                 name="f2ps") for _ in range(4)]
                w2_t = {}

                def ens2(fc2, dh=dh):
                    if fc2 < FC // 2 and fc2 not in w2_t:
                        w2_t[fc2] = wsp.tile([P, 2, 512], BF, tag="w2s",
                                             bufs=3, name="w2s")
                        [nc.gpsimd, nc.scalar][fc2 % 2].dma_start(
                            w2_t[fc2][:],
                            w2_batched[:, ds(2 * fc2, 2), ds(512 * dh, 512)])
                for fc2 in range(FC // 2):
                    ens2(fc2)
                    ens2(fc2 + 1)
                    w2t = w2_t[fc2]
                    for fi in range(2):
                        fc = 2 * fc2 + fi
                        for sl in range(4):
                            nc.tensor.matmul(
                                ops[sl][:], h1[:, fc, ts(sl, P)], w2t[:, fi],
                                start=(fc == 0), stop=(fc == FC - 1),
                            )
                    if filler is not None:
                        filler(fc2)
                for sl in range(4):
                    sc = half * 4 + sl
                    nc.vector.tensor_add(
                        out_sb[:, sc, ds(512 * dh, 512)], ops[sl][:],
                        res1[:, sc, ds(512 * dh, 512)],
                    )
                    nc.sync.dma_start(
                        out_r[sc][:, ds(512 * dh, 512)],
                        out_sb[:, sc, ds(512 * dh, 512)],
                    )

        with tc.tile_pool(name="wstream", bufs=1) as wsp:
            # attn0 with q-proj b1 steps as PE filler: scp 2x2 banks +
            # ctx2 2 + mixQ 2 = 8
            use_fa = os.environ.get("BASSK_FA", "1") == "1"
            use_fb = os.environ.get("BASSK_FB", "1") == "1"
            with tc.tile_pool(name="psAtt0", bufs=1, space="PSUM") as psT0:
                attn_half(psT0, 0,
                          filler=mk_fill_qproj(psT0) if use_fa else None)
            with tc.tile_pool(name="psMix0", bufs=1, space="PSUM") as psB0:
                wo_half(psB0, 0, bufs=4)
                n2Th0 = ln2_half(psB0, 0)
                if not use_fb:
                    h10 = ffn1_half(psB0, wsp, 0, n2Th0)
            # attn1 with ffn1(half 0) micro-ops as PE filler
            with tc.tile_pool(name="psAtt1", bufs=1, space="PSUM") as psT1:
                if use_fb:
                    h10, fillB, flushB = mk_fill_ffn1(psT1, wsp, n2Th0)
                    attn_half(psT1, 1, filler=fillB, exp_act_until=12)
                    flushB()
                else:
                    attn_half(psT1, 1)
            with tc.tile_pool(name="psMix1", bufs=1, space="PSUM") as psB1, \
                 tc.tile_pool(name="psF2", bufs=1, space="PSUM") as psF2:
                # wo(1) groups interleave into ffn2(0,dh0)'s PE queue (one
                # per two fc2 groups) so the wo/ln2(1) serial chains never
                # leave the PE idle; ln2(1)'s DVE work then hides under
                # ffn2(0,dh1)
                wo_seq = [(SC // 2 + sl, dh)
                          for sl in range(SC // 2) for dh in range(2)]

                def wo_fill(fc2):
                    if fc2 % 2 == 1 and wo_seq:
                        sc, dh = wo_seq.pop(0)
                        wo_group(psB1, sc, dh)
                ffn2_half(psF2, wsp, 0, h10, dhs=(0,), filler=wo_fill)
                while wo_seq:
                    sc, dh = wo_seq.pop(0)
                    wo_group(psB1, sc, dh)
                n2Th1 = ln2_half(psB1, 1)
                ffn2_half(psF2, wsp, 0, h10, dhs=(1,))
                h11 = ffn1_half(psB1, wsp, 1, n2Th1)
                ffn2_half(psF2, wsp, 1, h11)

    nc.compile()
    return nc


def _prep_inputs(inputs):
    f32 = lambda a: np.ascontiguousarray(np.asarray(a, dtype=np.float32))
    bfT = lambda a: np.ascontiguousarray(
        np.asarray(a, dtype=np.float32).T.astype(ml_dtypes.bfloat16))
    x = f32(inputs["x"])                      # [B, S, D]
    mask = np.asarray(inputs["src_mask"])     # [B, 1, 1, S] int32
    wqT = bfT(inputs["wq"])                   # [D, D] (in, out)
    woT = bfT(inputs["wo"])
    w1T = bfT(inputs["w1"])                   # [D, DFF]
    w2T = bfT(inputs["w2"])                   # [DFF, D]
    bq_v = np.ascontiguousarray(f32(inputs["bq"]).reshape(DC, P).T)
    b1_v = np.ascontiguousarray(f32(inputs["b1"]).reshape(FC, P).T)
    bo_rep = np.ascontiguousarray(
        np.tile(f32(inputs["bo"]), (P, 1)).astype(ml_dtypes.bfloat16))
    b2_rep = np.ascontiguousarray(
        np.tile(f32(inputs["b2"]), (P, 1)).astype(ml_dtypes.bfloat16))
    scal = lambda k: float(np.asarray(inputs[k]).reshape(-1)[0])
    ln = (scal("ln1_a"), scal("ln1_b"), scal("ln2_a"), scal("ln2_b"))
    mask_all_ones = bool((mask != 0).all())

    shared = dict(wqT=wqT, woT=woT, w1T=w1T, w2T=w2T, bq_v=bq_v, b1_v=b1_v,
                  bo_rep=bo_rep, b2_rep=b2_rep)
    in_maps = []
    for b in range(NB):
        m = dict(shared)
        m["x"] = np.ascontiguousarray(x[b])
        m["x_bf"] = np.ascontiguousarray(x[b].astype(ml_dtypes.bfloat16))
        if not mask_all_ones:
            m01 = (mask[b].reshape(S) != 0).astype(np.float32)
            m["m01_v"] = np.ascontiguousarray(m01.reshape(SC, P).T)
            m["m01_rep"] = np.ascontiguousarray(np.tile(m01, (P, 1)))
        in_maps.append(m)
    return in_maps, ln, mask_all_ones


last_nc = None
last_in_maps = None


def kernel(**inputs):
    global last_nc, last_in_maps
    in_maps, ln, mask_all_ones = _prep_inputs(inputs)
    nc = build_program(*ln, mask_all_ones)
    last_nc, last_in_maps = nc, in_maps
    res = bass_utils.run_bass_kernel_spmd(
        nc, in_maps, core_ids=list(range(NB)), trace=False,
    )
    out = np.stack([np.asarray(res.results[b]["out"]) for b in range(NB)])
    return out.astype(np.float32)



# revision 46
# speedup vs baseline: 1.0327x; 1.0327x over previous
"""Trainium2 Bass kernel for nn_EncoderBlock (dense transformer encoder block).

Strategy: pure data parallelism — batch B=8 across the 8 NeuronCores, one
batch element per core. No collectives. Per core:

  LN1 (bn_stats pairs over a bf16 x copy; DVE-only rsqrt via Taylor seed +
  Newton) -> q = n@wqT+bq (kh=vh=qh, reproducing the reference's q-reuse
  bug); attention per head pair, software-pipelined so scores(i+1) matmuls
  run under exp(i): S = q^T q (symmetric), E = exp(S/8 - 20);
  ctx AND Z come from ONE matmul per head: the stationary operand is the
  per-head slot [ones(64) | q(64)], so psum rows 0:64 hold Z (replicated)
  and rows 64:128 hold unnormalized ctx^T; normalization is one
  reciprocal_approx_fast + one cross-base tensor_mul per head (PSUM input
  may carry a different base partition than the SBUF operands — verified
  on HW).  The softmax exps alternate between ScalarE (exact ACT spline)
  and the DVE (Schraudolph bit-trick: i16 = round(x*128/ln2 + 127*128)
  bitcast to bf16, ~±3% on individual weights, cancels in the softmax
  ratio; measured no effect on final rel-err) so neither engine gates the
  scores->ctx pipeline; wo projection + residual; LN2; ReLU FFN (d_ff=4096)
  streamed from HBM on contention-separated queues; residual; out.

All transposes (n1T / qo / n2Th) are XBAR DMA-transposes — the PE does
only matmuls.  Matmuls run in bf16 (fp32 accumulation in PSUM); layernorm
statistics, softmax sums and the residual stream stay fp32.
"""

import sys

sys.path.insert(0, "/opt/trn_rl_repo")

import math

import numpy as np
import ml_dtypes
from contextlib import ExitStack

import concourse.bass as bass
import concourse.tile as tile
from concourse import bacc, mybir
from concourse import bass_utils
from concourse.bass import ts, ds
from concourse.masks import make_identity

BF = mybir.dt.bfloat16
F32 = mybir.dt.float32
I16 = mybir.dt.int16
AF = mybir.ActivationFunctionType
OP = mybir.AluOpType
AX = mybir.AxisListType

P = 128
S = 1024          # sequence length per core
D = 1024          # d_model
H = 16            # heads
DK = 64           # head dim
DFF = 4096
NB = 8            # batch = number of cores
SC = S // P       # 8 sequence chunks
DC = D // P       # 8 feature chunks
FC = DFF // P     # 32 ff chunks
EPS = 1e-6
EXP_SHIFT = -20.0  # constant shift inside exp; cancels in softmax ratio
# Schraudolph exp on the DVE: bf16 y with bits int16(round(x*128/ln2 +
# 127*128 - C)) satisfies y ~= exp(x)*(1 +/- 3%); C centers the one-sided
# piecewise-linear error.  Fold in the 1/8 score scale and EXP_SHIFT.
SCH_A = 0.125 * 128.0 / math.log(2.0)
SCH_B = 127.0 * 128.0 + EXP_SHIFT * (128.0 / math.log(2.0)) - 5.68

last_exec_time_ns = None


def _emit_layernorm(nc, small, xt, n_out, alpha, beta, idx, chunks,
                    apply_eng=None):
    """LN with Bessel-corrected std (ddof=1), matching torch/jax reference:
    n = (x - mu)/(std + eps)*alpha + beta.  xt [P,*,D] f32 indexed by `chunks`,
    n_out [P,len(chunks),D] bf16 indexed locally.
    Stats per token live on partitions; mean+var via one bn_stats pass (DVE
    only — keeps ScalarE free for the softmax exps)."""
    chunks = list(chunks)
    nch = len(chunks)
    BSD = nc.vector.BN_STATS_DIM
    bsf = 512  # BN_STATS_FMAX; D = 2 subgroups of 512
    nsub = D // bsf
    bst = small.tile([P, nch, nsub, BSD], F32, name=f"ln{idx}_bst")
    mv = small.tile([P, nch, 2], F32, name=f"ln{idx}_mv")
    var = small.tile([P, nch], F32, name=f"ln{idx}_var")
    tmp = small.tile([P, nch], F32, name=f"ln{idx}_tmp")
    tcoef = small.tile([P, nch], F32, name=f"ln{idx}_t")
    ucoef = small.tile([P, nch], F32, name=f"ln{idx}_u")

    for i, sc in enumerate(chunks):
        xv = xt[:, sc].rearrange("p (ns f) -> p ns f", ns=nsub)
        for sg in range(nsub):
            nc.vector.bn_stats(bst[:, i, sg], xv[:, sg])
        nc.vector.bn_aggr(mv[:, i], bst[:, i])
    mu = mv[:, :, 0]  # strided [P, nch] views
    # var (ddof=1)
    nc.vector.tensor_scalar_mul(var[:], mv[:, :, 1], float(D) / (D - 1))
    # 1/(std+eps) ~= rsqrt(var) (eps=1e-6 << std~1, relative error ~1e-6),
    # computed entirely on the DVE: LN variances concentrate near 1, so a
    # Taylor seed sqrt(r)~=1+(r-1)/2-(r-1)^2/8 off r=1/var (approx_fast)
    # plus two multiply-only Newton steps y*(1.5-0.5*var*y^2) reaches
    # ~1e-6 rel.  No ScalarE Sqrt -> no ACT-table switches anywhere.
    r = small.tile([P, nch], F32, name=f"ln{idx}_r")
    t2 = small.tile([P, nch], F32, name=f"ln{idx}_t2")
    y = small.tile([P, nch], F32, name=f"ln{idx}_y")
    nc.vector.reciprocal(r[:], var[:])
    nc.vector.tensor_scalar(tmp[:], r[:], 1.0, None, OP.subtract)  # t=r-1
    nc.vector.tensor_mul(t2[:], tmp[:], tmp[:])
    nc.vector.tensor_scalar(tmp[:], tmp[:], 0.5, 1.0, OP.mult, OP.add)
    nc.vector.scalar_tensor_tensor(y[:], t2[:], -0.125, tmp[:],
                                   OP.mult, OP.add)                # y0
    for _ in range(2):
        nc.vector.tensor_mul(t2[:], y[:], y[:])
        nc.vector.tensor_mul(t2[:], t2[:], var[:])
        nc.vector.tensor_scalar(t2[:], t2[:], -0.5, 1.5, OP.mult, OP.add)
        nc.vector.tensor_mul(y[:], y[:], t2[:])
    nc.vector.tensor_scalar_mul(tcoef[:], y[:], float(alpha))
    nc.vector.tensor_mul(tmp[:], mu, tcoef[:])
    nc.vector.tensor_scalar(ucoef[:], tmp[:], -1.0, float(beta), OP.mult, OP.add)
    apply_eng = apply_eng or nc.vector
    for i, sc in enumerate(chunks):
        if apply_eng is nc.scalar:
            # ACT affine: Identity(x*tcoef + ucoef) — rides the idle
            # ScalarE in the wo windows
            apply_eng.activation(
                n_out[:, i], xt[:, sc], mybir.ActivationFunctionType.Identity,
                bias=ucoef[:, ds(i, 1)], scale=tcoef[:, ds(i, 1)],
            )
        else:
            apply_eng.tensor_scalar(
                n_out[:, i], xt[:, sc], tcoef[:, ds(i, 1)], ucoef[:, ds(i, 1)],
                OP.mult, OP.add,
            )


def _emit_transpose(nc, pool, dst, src, ident, ca_range=range(8),
                    copy_engs=None):
    """dst = 8x8 block transpose of src; both [P, 8, 1024] (bf16).
    PSUM evacuations rotate over copy_engs (default Vector/Scalar split)."""
    for ca in ca_range:
        for cb in range(8):
            pt = pool.tile([P, P], src.dtype, tag="tp", bufs=4, name="tp")
            nc.tensor.transpose(pt[:], src[:, ca, ts(cb, P)], ident[:])
            engs = copy_engs or [nc.vector, nc.scalar]
            eng = engs[cb % len(engs)]
            if eng is nc.scalar:
                eng.copy(dst[:, cb, ts(ca, P)], pt[:])
            else:
                eng.tensor_copy(dst[:, cb, ts(ca, P)], pt[:])


def build_program(ln1a, ln1b, ln2a, ln2b, mask_all_ones):
    import os
    phase_stop = int(os.environ.get("BASSK_PHASE", "9"))
    nc = bacc.Bacc("TRN2", target_bir_lowering=False, debug=False)

    x_d = nc.dram_tensor("x", (S, D), F32, kind="ExternalInput").ap()
    xbf_d = nc.dram_tensor("x_bf", (S, D), BF, kind="ExternalInput").ap()
    wqT_d = nc.dram_tensor("wqT", (D, D), BF, kind="ExternalInput").ap()
    woT_d = nc.dram_tensor("woT", (D, D), BF, kind="ExternalInput").ap()
    w1T_d = nc.dram_tensor("w1T", (D, DFF), BF, kind="ExternalInput").ap()
    w2T_d = nc.dram_tensor("w2T", (DFF, D), BF, kind="ExternalInput").ap()
    bq_d = nc.dram_tensor("bq_v", (P, DC), F32, kind="ExternalInput").ap()
    b1_d = nc.dram_tensor("b1_v", (P, FC), F32, kind="ExternalInput").ap()
    bo_d = nc.dram_tensor("bo_rep", (P, D), BF, kind="ExternalInput").ap()
    b2_d = nc.dram_tensor("b2_rep", (P, D), BF, kind="ExternalInput").ap()
    if not mask_all_ones:
        m01_d = nc.dram_tensor("m01_v", (P, SC), F32, kind="ExternalInput").ap()
    out_d = nc.dram_tensor("out", (S, D), F32, kind="ExternalOutput").ap()

    x_r = x_d.rearrange("(sc p) d -> sc p d", p=P)
    xbf_r = xbf_d.rearrange("(sc p) d -> sc p d", p=P)
    wqT_r = wqT_d.rearrange("(kc p) o -> kc p o", p=P)
    woT_r = woT_d.rearrange("(oc p) d -> oc p d", p=P)
    w1_batched = w1T_d.rearrange("(dc p) f -> p dc f", p=P)
    w2_batched = w2T_d.rearrange("(fc p) d -> p fc d", p=P)
    out_r = out_d.rearrange("(sc p) d -> sc p d", p=P)

    with tile.TileContext(nc) as tc, ExitStack() as st:
        arena = st.enter_context(tc.tile_pool(name="arena", bufs=1))
        small = st.enter_context(tc.tile_pool(name="small", bufs=1))

        # ---- constants ----
        ident_b = small.tile([P, P], BF, name="ident_b")
        make_identity(nc, ident_b[:])
        ebias = small.tile([P, 1], F32, name="ebias")
        nc.gpsimd.memset(ebias[:], EXP_SHIFT)
        # warm the GpSimd tensor-op ucode library at t=0: its ~6us IRAM
        # load otherwise lands right on the LN1-apply critical path
        gpwarm = small.tile([P, 1], F32, name="gpwarm")
        nc.gpsimd.tensor_scalar(gpwarm[:], ebias[:], 0.0, None, OP.mult)
        bq_sb = small.tile([P, DC], F32, name="bq_sb")
        b1_sb = small.tile([P, FC], F32, name="b1_sb")
        bo_rep = small.tile([P, D], BF, name="bo_rep")
        b2_rep = small.tile([P, D], BF, name="b2_rep")
        if not mask_all_ones:
            m01_sb = small.tile([P, SC], F32, name="m01_sb")
            nc.sync.dma_start(m01_sb[:], m01_d)

        def emit_bias_dmas():
            # biases are needed late; ScalarE has early-queue slack and
            # keeping them off Sync/GpSimd unblocks the n1T transposes and
            # the LN1 applies
            nc.scalar.dma_start(bq_sb[:], bq_d)
            nc.scalar.dma_start(b1_sb[:], b1_d)
            nc.scalar.dma_start(bo_rep[:], bo_d)
            nc.scalar.dma_start(b2_rep[:], b2_d)

        dma_engines = [nc.sync, nc.scalar, nc.gpsimd]
        # DMA issue queues that never carry softmax exps — weight streams
        # during the attention/FFN overlap must not block the Scalar queue
        # (DMA_DIRECT2D there waits on FFN semaphores ahead of exps).
        dma_quiet = [nc.sync, nc.gpsimd]

        # ---- phase A inputs ----
        # The startup is chip-HBM-contention-bound (all 8 cores load at
        # once), so LN1 reads a bf16 copy of x — half the bytes.  The f32 x
        # is only needed for the residual and is re-loaded later, off the
        # critical path.  GpSimd carries no weight DMAs so the LN applies
        # are not queued behind arrivals.
        xt = arena.tile([P, SC, D], BF, tag="xt_h1", name="xt")
        # keep GpSimd's early queue free for the LN1 applies
        x_eng = [nc.sync, nc.scalar, nc.sync, nc.scalar,
                 nc.sync, nc.scalar, nc.sync, nc.scalar]
        for sc in range(SC):
            x_eng[sc].dma_start(xt[:, sc], xbf_r[sc])
        qT = arena.tile([P, DC, S], BF, tag="qT", name="qT")  # [o%P, oc, s]
        # Fused ctx+Z stationary slots: qo[:, hp, c, h] = [ones(64)|q_h(64)]
        # so ONE matmul per (head, chunk) yields Z (psum rows 0:64,
        # replicated) and unnormalized ctx^T (rows 64:128).  Layout is
        # hp-major with c-stride == 2*h-stride so one DMA-transpose per oc
        # fills all 4 chunks x 2 heads (the (c,h) dims merge to a legal 3D
        # dst); the ones halves are memset once.
        qo = arena.tile([P, DC, SC, 2, P], BF, tag="qq_out", name="qo")
        n1 = arena.tile([P, SC, D], BF, tag="n1_ctx", name="n1")
        n1T = arena.tile([P, DC, S], BF, tag="n1T_woT", name="n1T")
        wq_sb = arena.tile([P, DC, D], BF, tag="wq_res1", name="wq_sb")
        for kc in range(DC):
            [nc.sync, nc.scalar][kc % 2].dma_start(wq_sb[:, kc], wqT_r[kc])
        emit_bias_dmas()

        # ================= phase A: LN1, q projection, transposes ============
        # LN1 split into halves so the first q-proj matmuls (which only read
        # n1T columns 0:512 = seq chunks 0-3) start while LN of chunks 4-7
        # still runs on the DVE.
        with tc.tile_pool(name="psA", bufs=1, space="PSUM") as psA:
            # HAM warm-up: dead transposes keep the PE clock-gate's busy
            # window active across the whole LN1 window (~15-30us of
            # HBM-contended arrivals), so q-proj and early attention run at
            # 2.4 GHz instead of starting cold at 1.2 GHz
            for _ in range(120):
                wup = psA.tile([P, P], BF, tag="tp", bufs=4, name="wup")
                nc.tensor.transpose(wup[:], ident_b[:], ident_b[:])
            # LN1 in chunk pairs: stats on Vector, applies on GpSimd, and
            # the n1 -> n1T transposes as XBAR DMA-transposes on Sync (one
            # [128,1024] DMA per chunk writes the whole strided slice) —
            # three disjoint queues, nothing blocks the next pair's stats,
            # and the PE does no transpose work at all.
            for pi in range(SC // 2):
                pr = range(2 * pi, 2 * pi + 2)
                _emit_layernorm(nc, small, xt, n1[:, ds(2 * pi, 2)],
                                ln1a, ln1b, f"1p{pi}", pr,
                                apply_eng=nc.gpsimd)
                # NB: all DMA-transposes must stay on ONE queue — the
                # transpose XBAR is a shared resource; driving it from two
                # queues concurrently corrupts the output (measured).
                for ca in pr:
                    nc.sync.dma_start_transpose(
                        n1T[:, :, ts(ca, P)], n1[:, ca])

            def qproj_oc(pool, tag, bufs, b, oc, evac_act=False):
                pbt = pool.tile([P, 512], F32, tag=tag, bufs=bufs,
                                name="qps")
                for kc in range(DC):
                    nc.tensor.matmul(
                        pbt[:], wq_sb[:, kc, ts(oc, P)],
                        n1T[:, kc, ds(512 * b, 512)],
                        start=(kc == 0), stop=(kc == DC - 1),
                    )
                if evac_act:
                    # prolog evacs ride ScalarE (Identity+bias) so the DVE
                    # keeps its attention exp budget
                    nc.scalar.add(
                        qT[:, oc, ds(512 * b, 512)], pbt[:],
                        bq_sb[:, ds(oc, 1)],
                    )
                else:
                    nc.vector.tensor_scalar(
                        qT[:, oc, ds(512 * b, 512)], pbt[:],
                        bq_sb[:, ds(oc, 1)], None, OP.add,
                    )

            for oc in range(DC):
                qproj_oc(psA, "qps", 4, 0, oc)
            # ones halves of the qo slots, in small chunks emitted after
            # the qproj evacs: the Tile priority heap then only runs them
            # in DVE idle slots (one 7us memset here measurably delayed the
            # whole LN1 -> qproj critical path)
            qo_ones = qo[:].rearrange("p a b c d -> p (a b c) d")[:, :, 0:DK]
            for oc in range(DC):
                nc.vector.memset(qo_ones[:, ds(16 * oc, 16)], 1.0)

            def qo_transpose(oc):
                # One [64,1024] XBAR DMA-transpose per head fills all 8
                # chunks of that head's qo slots: dst [p, c(8), d(64)] has
                # mid extent == transposed row blocks and last extent ==
                # transposed cols (the only shape the engine honors).
                # Requires qT[:, oc, :] complete (b0 AND b1).
                for hl in range(2):
                    nc.sync.dma_start_transpose(
                        qo[:, oc, :, hl, DK:P],
                        qT[ds(hl * DK, DK), oc, :])

            # q-proj b1 for oc 0,1 now (their qo slots are needed in
            # attn0's first iterations — too early for the spread-out
            # filler steps); the rest of b1 spreads across attn0
            n_pre = 2 if os.environ.get("BASSK_FA", "1") == "1" else DC
            for oc in range(n_pre):
                qproj_oc(psA, "qps", 4, 1, oc)
                qo_transpose(oc)

        def mk_fill_qproj(pool):
            # Spread q-proj b1 (oc 2..7) across attn0's iterations: one
            # 512-cycle contraction step per filler call keeps the PE queue
            # dense underneath the exp latency (no stall, HAM stays warm).
            st_ = {}
            steps = []
            for oc in range(2, DC):
                for kc in range(DC):
                    def step(oc=oc, kc=kc):
                        if kc == 0:
                            st_[oc] = pool.tile([P, 512], F32, tag="mixQ",
                                                bufs=2, name="qps1")
                        nc.tensor.matmul(
                            st_[oc][:], wq_sb[:, kc, ts(oc, P)],
                            n1T[:, kc, ds(512, 512)],
                            start=(kc == 0), stop=(kc == DC - 1),
                        )
                        if kc == DC - 1:
                            nc.scalar.add(
                                qT[:, oc, ds(512, 512)], st_[oc][:],
                                bq_sb[:, ds(oc, 1)])
                            qo_transpose(oc)
                    steps.append(step)
            done = [0]

            def filler(idx):
                target = min(len(steps), (idx + 1) * len(steps) // 50)
                while done[0] < target:
                    steps[done[0]]()
                    done[0] += 1
            return filler

        if phase_stop <= 1:
            for sc in range(SC):
                dt_ = arena.tile([P, D], F32, tag="dump", bufs=2, name="dump")
                nc.vector.tensor_copy(dt_[:], qT[:, sc])
                nc.sync.dma_start(out_r[sc], dt_[:])
            nc.compile()
            return nc
        if phase_stop == 2:
            for sc in range(SC):
                dt_ = arena.tile([P, D], F32, tag="dump", bufs=2, name="dump")
                nc.vector.tensor_copy(
                    dt_[:].rearrange("p (b c d) -> p b c d", b=4, c=2),
                    qo[:, sc // 2, (sc % 2) * 4:(sc % 2) * 4 + 4])
                nc.sync.dma_start(out_r[sc], dt_[:])
            nc.compile()
            return nc

        # persistent across the pipelined halves
        ctxT = arena.tile([P, DC, S], BF, tag="n1_ctx", name="ctxT")
        woT_sb = arena.tile([P, DC, D], BF, tag="n1T_woT", name="woT_sb")
        # anchor the woT stream behind qT so the scheduler cannot hoist
        # these 2MB of loads into the startup HBM window (x_bf/wq arrival
        # paces the whole LN1->qproj chain)
        nc.scalar.copy(woT_sb[0:1, 0, 0:1], qT[0:1, 0, 0:1])
        for oc in range(DC):
            nc.sync.dma_start(woT_sb[:, oc], woT_r[oc])
        res1 = arena.tile([P, SC, D], F32, tag="wq_res1", name="res1")
        out_sb = None

        # ============ attention / wo / LN2 / FFN pipelined by query halves ===
        #
        # The 128 softmax exps alternate ScalarE (exact) / DVE (Schraudolph
        # int16 bit-trick written through a bf16 bitcast view) so no single
        # engine gates the pipeline; the fused [ones|q] stationary gives
        # ctx and Z from one matmul per (head, chunk).

        def attn_half(psT, half, filler=None, look=1, sbufs=2,
                      exp_act_until=0):
            # Software-pipelined: scores(i+look) matmuls are EMITTED before
            # ctx(i) so the in-order PE queue runs them underneath exp(i);
            # `filler(idx)` injects independent PE work (q-proj b1 steps,
            # ffn1 chunks) between scores(i+look) and ctx(i) — the queue
            # stays dense so the exp latency never stalls the PE and the
            # HAM clock-gate stays at 2.4 GHz.  The exps split 5:3
            # ScalarE:DVE — the DVE also carries the normalize.
            iters = [(hp, c) for hp in range(H // 2) for c in range(SC)]
            state = {}

            def emit_scores(hp, c):
                sp = psT.tile([P, 1024], F32, tag="scp", bufs=sbufs,
                              name="scp")
                for hl in range(2):
                    lo = hl * DK
                    nc.tensor.matmul(
                        sp[:, ds(hl * 512, 512)],
                        qT[ds(lo, DK), hp, ts(c, P)],
                        qT[ds(lo, DK), hp, ds(512 * half, 512)],
                        start=True, stop=True,
                        tile_position=(lo, 0),
                    )
                return sp

            sp_pend = [emit_scores(*iters[i]) for i in range(look)]
            for idx, (hp, c) in enumerate(iters):
                sp = sp_pend.pop(0)
                if idx + look < len(iters):
                    sp_pend.append(emit_scores(*iters[idx + look]))
                if filler is not None:
                    filler(idx)
                if hp not in state:
                    state[hp] = psT.tile([P, 2 * 512], F32, tag="ctx2",
                                         bufs=1, name="ctx2")
                ctx2 = state[hp]
                # ec is consumed by the two fused matmuls of this iteration
                # only — a rolling buffer is enough pipeline depth
                ec = arena.tile([P, 2 * 512], BF, tag="EC", bufs=4,
                                name="ec")
                if idx < exp_act_until or (idx % 8) not in (1, 4, 6):
                    nc.scalar.activation(
                        ec[:], sp[:], AF.Exp, bias=ebias[:], scale=0.125,
                    )
                else:
                    nc.vector.tensor_scalar(
                        ec[:].bitcast(I16), sp[:], SCH_A, SCH_B,
                        OP.mult, OP.add,
                    )
                if not mask_all_ones:
                    nc.vector.tensor_scalar_mul(
                        ec[:], ec[:], m01_sb[:, ds(c, 1)],
                    )
                for hl in range(2):
                    nc.tensor.matmul(
                        ctx2[:, ds(hl * 512, 512)], qo[:, hp, c, hl],
                        ec[:, ds(hl * 512, 512)],
                        start=(c == 0), stop=(c == SC - 1),
                        skip_group_check=True,
                    )
                if c == SC - 1:
                    # Z sits on psum rows 0:64 (base 0 — required by
                    # reciprocal_approx_fast), ctx^T on rows 64:128; the
                    # muls read the PSUM ctx at base 64 against base-0 SBUF
                    # operands (HW-verified legal: only SBUF-SBUF operand
                    # pairs must share a base partition).
                    rz = arena.tile([DK, 2 * 512], F32, tag="rz", bufs=1,
                                    name="rz")
                    nc.vector.reciprocal_approx_fast(rz[:], ctx2[0:DK])
                    nc.vector.tensor_mul(
                        ctxT[0:DK, hp, ds(512 * half, 512)],
                        ctx2[DK:P, ds(0, 512)], rz[:, ds(0, 512)])
                    nc.vector.tensor_mul(
                        ctxT[DK:P, hp, ds(512 * half, 512)],
                        ctx2[DK:P, ds(512, 512)], rz[:, ds(512, 512)])
                    del state[hp]

        def wo_group(psW, sc, dh, bufs=2):
            if dh == 0:
                xre = arena.tile([P, D], F32, tag="xre", bufs=2, name="xre")
                wo_group.xre[sc] = xre
                # same anti-hoist anchor as woT: keep the 512KB f32 x loads
                # out of the startup window
                nc.scalar.copy(xre[0:1, 0:1], qT[0:1, 0, 0:1])
                nc.sync.dma_start(xre[:], x_r[sc])
                # precombine x + bo off the critical path so each wo PSUM
                # bank frees after a single add
                nc.gpsimd.tensor_add(xre[:], xre[:], bo_rep[:])
            xre = wo_group.xre[sc]
            wp = psW.tile([P, 512], F32, tag="mix", bufs=bufs, name="wops")
            for oc in range(DC):
                nc.tensor.matmul(
                    wp[:], ctxT[:, oc, ts(sc, P)],
                    woT_sb[:, oc, ds(512 * dh, 512)],
                    start=(oc == 0), stop=(oc == DC - 1),
                )
            nc.vector.tensor_add(
                res1[:, sc, ds(512 * dh, 512)], wp[:],
                xre[:, ds(512 * dh, 512)],
            )
        wo_group.xre = {}

        def wo_half(psW, half, bufs=2):
            for sl in range(SC // 2):
                for dh in range(2):
                    wo_group(psW, half * (SC // 2) + sl, dh, bufs=bufs)

        def ln2_half(psB, half):
            n2h = arena.tile([P, SC // 2, D], BF, tag="n2h", bufs=1,
                             name="n2h")
            chunks = range(half * (SC // 2), (half + 1) * (SC // 2))
            _emit_layernorm(nc, small, res1, n2h, ln2a, ln2b, f"2h{half}",
                            chunks,
                            apply_eng=nc.scalar if half == 1 else nc.vector)
            n2Th = arena.tile([P, DC, 512], BF, tag="n2th", bufs=1,
                              name="n2Th")
            for ca in range(SC // 2):
                nc.sync.dma_start_transpose(n2Th[:, :, ts(ca, P)],
                                            n2h[:, ca])
            for sc in chunks:
                nc.gpsimd.tensor_add(res1[:, sc], res1[:, sc], b2_rep[:])
            return n2Th

        def ffn1_half(psB, wsp, half, n2Th):
            h1 = arena.tile([P, FC, 512], BF, tag="xt_h1", name="h1")
            # two queues hide the per-DMA completion latency.  Half 0 runs
            # concurrently with attention-half-1 exps, so its second queue
            # is Sync (Scalar would stall exps behind buffer-gated DMAs);
            # half 1 runs when Scalar is exp-free.
            w1q = [nc.gpsimd, nc.sync if half == 0 else nc.scalar]
            wts_t = {}

            def ens1(fc):
                if fc < FC and fc not in wts_t:
                    wts_t[fc] = wsp.tile([P, DC, P], BF, tag="w1s", bufs=3,
                                         name="w1s")
                    w1q[fc % 2].dma_start(wts_t[fc][:],
                                          w1_batched[:, :, ts(fc, P)])
            for fc in range(FC):
                ens1(fc)
                ens1(fc + 1)
                wts = wts_t[fc]
                fp = psB.tile([P, 512], F32, tag="mix", bufs=2, name="f1ps")
                for dc in range(DC):
                    nc.tensor.matmul(
                        fp[:], wts[:, dc], n2Th[:, dc, :],
                        start=(dc == 0), stop=(dc == DC - 1),
                    )
                nc.vector.tensor_scalar(
                    h1[:, fc], fp[:], b1_sb[:, ds(fc, 1)], 0.0,
                    OP.add, OP.max,
                )
            return h1

        def mk_fill_ffn1(psT, wsp, n2Th, start_iter=12):
            # ffn1(half 0) sliced into 2-matmul micro-ops and fed into
            # attn1's filler slots: the independent FFN work absorbs the
            # exp latency and keeps the PE at 2.4 GHz.  Starts a few
            # iterations in so ln2(0)'s DVE work and n2Th transposes clear
            # first.  w1 streams are issued one fc ahead of their matmuls.
            h1 = arena.tile([P, FC, 512], BF, tag="xt_h1", name="h1")
            st_ = {}
            w1q = [nc.gpsimd, nc.sync]

            def ensure_dma(fc):
                if fc < FC and fc not in st_:
                    wts = wsp.tile([P, DC, P], BF, tag="w1s", bufs=3,
                                   name="w1s")
                    w1q[fc % 2].dma_start(wts[:],
                                          w1_batched[:, :, ts(fc, P)])
                    st_[fc] = wts

            units = []
            for fc in range(FC):
                for u in range(4):
                    def unit(fc=fc, u=u):
                        if u == 0:
                            ensure_dma(fc)
                            ensure_dma(fc + 1)
                            st_[fc] = (st_[fc],
                                       psT.tile([P, 512], F32, tag="mixF",
                                                bufs=2, name="f1ps"))
                        wts, fp = st_[fc]
                        for dc in (2 * u, 2 * u + 1):
                            nc.tensor.matmul(
                                fp[:], wts[:, dc], n2Th[:, dc, :],
                                start=(dc == 0), stop=(dc == DC - 1),
                            )
                        if u == 3:
                            # evacs alternate ScalarE (Relu+bias via the
                            # ACT affine) / DVE so neither engine's exp
                            # budget is eaten
                            if fc % 2 == 0:
                                nc.scalar.activation(
                                    h1[:, fc], fp[:], AF.Relu,
                                    bias=b1_sb[:, ds(fc, 1)], scale=1.0)
                            else:
                                nc.vector.tensor_scalar(
                                    h1[:, fc], fp[:], b1_sb[:, ds(fc, 1)],
                                    0.0, OP.add, OP.max)
                    units.append(unit)
            done = [0]

            def filler(idx):
                if idx < start_iter:
                    return
                navail = 64 - start_iter - 2
                target = min(len(units),
                             (idx - start_iter + 1) * len(units) // navail)
                while done[0] < target:
                    units[done[0]]()
                    done[0] += 1

            def flush():
                while done[0] < len(units):
                    units[done[0]]()
                    done[0] += 1
            return h1, filler, flush

        def ffn2_half(psF2, wsp, half, h1, dhs=(0, 1), filler=None):
            nonlocal out_sb
            if out_sb is None:
                out_sb = arena.tile([P, SC, D], F32, tag="qq_out",
                                    name="out_sb")
            for dh in dhs:
                ops = [psF2.tile([P, 512], F32, tag="f2ps", bufs=6,
                                 name="f2ps") for _ in range(4)]
                w2_t = {}

                def ens2(fc2, dh=dh):
                    if fc2 < FC // 2 and fc2 not in w2_t:
                        w2_t[fc2] = wsp.tile([P, 2, 512], BF, tag="w2s",
                                             bufs=3, name="w2s")
                        [nc.gpsimd, nc.scalar][fc2 % 2].dma_start(
                            w2_t[fc2][:],
                            w2_batched[:, ds(2 * fc2, 2), ds(512 * dh, 512)])
                for fc2 in range(FC // 2):
                    ens2(fc2)
                    ens2(fc2 + 1)
                    w2t = w2_t[fc2]
                    for fi in range(2):
                        fc = 2 * fc2 + fi
                        for sl in range(4):
                            nc.tensor.matmul(
                                ops[sl][:], h1[:, fc, ts(sl, P)], w2t[:, fi],
                                start=(fc == 0), stop=(fc == FC - 1),
                            )
                    if filler is not None:
                        filler(fc2)
                for sl in range(4):
                    sc = half * 4 + sl
                    nc.vector.tensor_add(
                        out_sb[:, sc, ds(512 * dh, 512)], ops[sl][:],
                        res1[:, sc, ds(512 * dh, 512)],
                    )
                    nc.sync.dma_start(
                        out_r[sc][:, ds(512 * dh, 512)],
                        out_sb[:, sc, ds(512 * dh, 512)],
                    )

        with tc.tile_pool(name="wstream", bufs=1) as wsp:
            # attn0 with q-proj b1 steps as PE filler: scp 2x2 banks +
            # ctx2 2 + mixQ 2 = 8
            use_fa = os.environ.get("BASSK_FA", "1") == "1"
            use_fb = os.environ.get("BASSK_FB", "1") == "1"
            with tc.tile_pool(name="psAtt0", bufs=1, space="PSUM") as psT0:
                attn_half(psT0, 0,
                          filler=mk_fill_qproj(psT0) if use_fa else None)
            with tc.tile_pool(name="psMix0", bufs=1, space="PSUM") as psB0:
                wo_half(psB0, 0, bufs=4)
                n2Th0 = ln2_half(psB0, 0)
                if not use_fb:
                    h10 = ffn1_half(psB0, wsp, 0, n2Th0)
            # attn1 with ffn1(half 0) micro-ops as PE filler
            with tc.tile_pool(name="psAtt1", bufs=1, space="PSUM") as psT1:
                if use_fb:
                    h10, fillB, flushB = mk_fill_ffn1(psT1, wsp, n2Th0)
                    attn_half(psT1, 1, filler=fillB, exp_act_until=12)
                    flushB()
                else:
                    attn_half(psT1, 1)
            with tc.tile_pool(name="psMix1", bufs=1, space="PSUM") as psB1, \
                 tc.tile_pool(name="psF2", bufs=1, space="PSUM") as psF2:
                # wo(1) groups interleave into ffn2(0,dh0)'s PE queue (one
                # per two fc2 groups) so the wo/ln2(1) serial chains never
                # leave the PE idle; ln2(1)'s DVE work then hides under
                # ffn2(0,dh1)
                wo_seq = [(SC // 2 + sl, dh)
                          for sl in range(SC // 2) for dh in range(2)]

                def wo_fill(fc2):
                    if fc2 % 2 == 1 and wo_seq:
                        sc, dh = wo_seq.pop(0)
                        wo_group(psB1, sc, dh)
                ffn2_half(psF2, wsp, 0, h10, dhs=(0,), filler=wo_fill)
                while wo_seq:
                    sc, dh = wo_seq.pop(0)
                    wo_group(psB1, sc, dh)
                n2Th1 = ln2_half(psB1, 1)
                ffn2_half(psF2, wsp, 0, h10, dhs=(1,))
                h11 = ffn1_half(psB1, wsp, 1, n2Th1)
                ffn2_half(psF2, wsp, 1, h11)

    nc.compile()
    return nc


def _prep_inputs(inputs):
    f32 = lambda a: np.ascontiguousarray(np.asarray(a, dtype=np.float32))
    bfT = lambda a: np.ascontiguousarray(
        np.asarray(a, dtype=np.float32).T.astype(ml_dtypes.bfloat16))
    x = f32(inputs["x"])                      # [B, S, D]
    mask = np.asarray(inputs["src_mask"])     # [B, 1, 1, S] int32
    wqT = bfT(inputs["wq"])                   # [D, D] (in, out)
    woT = bfT(inputs["wo"])
    w1T = bfT(inputs["w1"])                   # [D, DFF]
    w2T = bfT(inputs["w2"])                   # [DFF, D]
    bq_v = np.ascontiguousarray(f32(inputs["bq"]).reshape(DC, P).T)
    b1_v = np.ascontiguousarray(f32(inputs["b1"]).reshape(FC, P).T)
    bo_rep = np.ascontiguousarray(
        np.tile(f32(inputs["bo"]), (P, 1)).astype(ml_dtypes.bfloat16))
    b2_rep = np.ascontiguousarray(
        np.tile(f32(inputs["b2"]), (P, 1)).astype(ml_dtypes.bfloat16))
    scal = lambda k: float(np.asarray(inputs[k]).reshape(-1)[0])
    ln = (scal("ln1_a"), scal("ln1_b"), scal("ln2_a"), scal("ln2_b"))
    mask_all_ones = bool((mask != 0).all())

    shared = dict(wqT=wqT, woT=woT, w1T=w1T, w2T=w2T, bq_v=bq_v, b1_v=b1_v,
                  bo_rep=bo_rep, b2_rep=b2_rep)
    in_maps = []
    for b in range(NB):
        m = dict(shared)
        m["x"] = np.ascontiguousarray(x[b])
        m["x_bf"] = np.ascontiguousarray(x[b].astype(ml_dtypes.bfloat16))
        if not mask_all_ones:
            m01 = (mask[b].reshape(S) != 0).astype(np.float32)
            m["m01_v"] = np.ascontiguousarray(m01.reshape(SC, P).T)
            m["m01_rep"] = np.ascontiguousarray(np.tile(m01, (P, 1)))
        in_maps.append(m)
    return in_maps, ln, mask_all_ones


last_nc = None
last_in_maps = None


def kernel(**inputs):
    global last_nc, last_in_maps
    in_maps, ln, mask_all_ones = _prep_inputs(inputs)
    nc = build_program(*ln, mask_all_ones)
    last_nc, last_in_maps = nc, in_maps
    res = bass_utils.run_bass_kernel_spmd(
        nc, in_maps, core_ids=list(range(NB)), trace=False,
    )
    out = np.stack([np.asarray(res.results[b]["out"]) for b in range(NB)])
    return out.astype(np.float32)

